# revision 1
# baseline (speedup 1.0000x reference)
"""Trainium2 kernel for BinaryXnorExceptOutliersLinear.

Computes  out = x @ w_sim.T + bias  where
  w_sim = where(outlier_mask, weight, sign(weight) * binary_scale)

Distribution: column-parallel over 8 NeuronCores — weight / outlier_mask /
bias are sharded along out_features (11008 -> 8 x 1376), x is replicated,
each core produces its [8192, 1376] output slice, concatenated on host.

Per-core kernel:
  1. Weight prep (once): DMA weight+mask shard, sign via ACT, scale via DVE
     tensor_scalar, outlier restore via DVE copy_predicated, then PE
     transpose into a SBUF-resident [K=4096, 1376] bf16 wT (88KB/partition).
  2. Main loop over 64 token tiles: gpsimd cast-DMA x f32->bf16, PE
     transposes x tiles (interleaved with the previous tile's matmuls so PE
     stays busy), 3x32 accumulating matmuls per tile (chunks 512/512/352
     over out-features, 32 k-tiles), DVE adds bias on the PSUM->SBUF copy,
     DMA out.
"""

import sys

for _p in ("/opt/trn_rl_repo",):
    if _p not in sys.path:
        sys.path.insert(0, _p)

import ml_dtypes
import numpy as np

import concourse.bass as bass
import concourse.mybir as mybir
from concourse.tile import TileContext
from concourse.bass_utils import run_bass_kernel_spmd

B, S, DIN, DOUT = 4, 2048, 4096, 11008
M = B * S              # 8192 tokens
NCORES = 8
DSH = DOUT // NCORES   # 1376 out-features per core
K = DIN
KT = K // 128          # 32 k-tiles
CHUNKS = [(0, 512), (512, 512), (1024, 352)]   # out-feature chunks per core

F32 = mybir.dt.float32
BF16 = mybir.dt.bfloat16
U8 = mybir.dt.uint8

MAX_WAITS = 1  # stock walrus: one sem-wait command per instruction


def _split_excess_waits(nc, max_waits: int = MAX_WAITS) -> int:
    """Stock AWS walrus rejects instructions with more than one sem-wait
    ("Too many sync wait commands"). Tile's kernel-tail drain waits on the
    final value of every proc's semaphore. Peel excess waits onto bare
    EventSemaphore stubs placed right before the instruction on the same
    engine (engines run their stream in order, so ordering is preserved)."""
    n_split = 0
    for f in nc.m.functions:
        for blk in f.blocks:
            il = blk.instructions
            out = []
            changed = False
            for inst in il:
                si = inst.sync_info
                waits = list(si.on_wait) if (si and si.on_wait) else []
                if len(waits) > max_waits:
                    changed = True
                    extra, keep = waits[:-max_waits], waits[-max_waits:]
                    for ci, start in enumerate(range(0, len(extra), max_waits)):
                        chunk = extra[start:start + max_waits]
                        stub = mybir.InstEventSemaphore(
                            name=f"{inst.name}_wsplit{ci}", ins=[], outs=[])
                        stub.engine = inst.engine
                        stub.sync_info = mybir.SyncInfo(
                            on_wait=list(chunk), on_update=[])
                        out.append(stub)
                        n_split += 1
                    si.on_wait = keep
                    inst.sync_info = si
                out.append(inst)
            if changed:
                il.clear()
                il.extend(out)
    return n_split


def build_nc(m_tokens: int = M):
    """Build the per-core Bass program (SPMD: same program on all cores)."""
    tok_tiles = m_tokens // 128
    nc = bass.Bass()
    x_h = nc.declare_dram_parameter("x", [m_tokens, K], F32, isOutput=False)
    w_h = nc.declare_dram_parameter("weight", [DSH, K], F32, isOutput=False)
    b_h = nc.declare_dram_parameter("bias", [DSH], F32, isOutput=False)
    mk_h = nc.declare_dram_parameter("outlier_mask", [DSH, K], U8, isOutput=False)
    sc_h = nc.declare_dram_parameter("binary_scale", [1, 1], F32, isOutput=False)
    id_h = nc.declare_dram_parameter("identity_const", [128, 128], BF16,
                                     isOutput=False)
    out_h = nc.declare_dram_parameter("out", [m_tokens, DSH], F32, isOutput=True)

    dout_tiles = [(o, min(128, DSH - o)) for o in range(0, DSH, 128)]

    with TileContext(nc) as tc:
        with tc.tile_pool(name="const", bufs=1) as const_pool:

            identity = const_pool.tile([128, 128], BF16)
            nc.scalar.dma_start(identity, id_h[:, :])
            scale_vec = const_pool.tile([128, 1], F32)
            nc.gpsimd.dma_start(out=scale_vec,
                                in_=sc_h[:, :].to_broadcast((128, 1)))
            bias_rep = const_pool.tile([128, DSH], F32)

            # Resident binarized+transposed weight: [k-in-tile, kt, dout]
            wT = const_pool.tile([128, KT * DSH], BF16)
            wT_r = wT.rearrange("p (kt d) -> p kt d", kt=KT)

            # Emission (= engine program) order:
            #   1. prologue: x transposes for the first PRO token tiles —
            #      PE has work within microseconds of kernel start,
            #   2. weight prep (its DVE chain overlaps the prologue),
            #   3. steady state: x transposes for tile t+PRO interleaved
            #      per-k-tile with tile t's matmuls, so the PE never idles.
            PRO = 3
            with tc.tile_pool(name="xmain", bufs=2) as xp, \
                 tc.tile_pool(name="xtp", bufs=PRO + 1) as xtp, \
                 tc.tile_pool(name="wprep", bufs=2) as wp, \
                 tc.tile_pool(name="mpsum", bufs=2, space="PSUM") as psum_pool:

                xTs = {}

                def x_transpose(t, kt, psx_box):
                    """Emit transpose of x tile t, k-tile kt (+psum staging)."""
                    j = kt % 4
                    if kt == 0:
                        xb = xp.tile([128, K], BF16, tag="xb", name="xb")
                        # split the cast-DMA so the first transposes start
                        # after a fraction of the tile has landed (finest
                        # split on the very first tiles to cut startup idle)
                        nq = 16 if t == 0 else (8 if t == 1 else 4)
                        for q in range(nq):
                            qs = slice(q * (K // nq), (q + 1) * (K // nq))
                            nc.gpsimd.dma_start(
                                xb[:, qs], x_h[t * 128:(t + 1) * 128, qs])
                        xTs[t] = (xtp.tile([128, K], BF16, tag="xT", name="xT"),
                                  xb)
                    xT, xb = xTs[t]
                    if j == 0:
                        psx_box[0] = psum_pool.tile([128, 512], BF16,
                                                    tag="psx", name="psx")
                    nc.tensor.transpose(
                        psx_box[0][:, j * 128:(j + 1) * 128],
                        xb[:, kt * 128:(kt + 1) * 128], identity)
                    if j == 3:
                        nc.vector.tensor_copy(
                            xT[:, (kt - 3) * 128:(kt + 1) * 128], psx_box[0])

                box = [None]
                # prologue transpose work in 4-k-tile groups (groups must
                # stay contiguous since psw shares the psx tag); emitted
                # around weight-prep stages so the PE has independent work
                # while the DVE chain runs
                pro_chunks = [(t, g4) for t in range(min(PRO, tok_tiles))
                              for g4 in range(KT // 4)]

                def emit_pro(n):
                    for _ in range(n):
                        if not pro_chunks:
                            return
                        t, g4 = pro_chunks.pop(0)
                        for kt in range(g4 * 4, g4 * 4 + 4):
                            x_transpose(t, kt, box)

                psos_map = {}

                def emit_mm(t, ci):
                    """Emit the 32-matmul accumulation for (token tile, chunk)
                    as soon as its wT columns are ready (staged into the
                    weight-prep region to fill PE stalls)."""
                    coff, csz = CHUNKS[ci]
                    ps = psum_pool.tile([128, 512], F32, tag=f"pso{ci}",
                                        name=f"pso{ci}")
                    psos_map[(t, ci)] = ps
                    xT_t = xTs[t][0]
                    for kt in range(KT):
                        nc.tensor.matmul(
                            ps[:, :csz], xT_t[:, kt * 128:(kt + 1) * 128],
                            wT_r[:, kt, coff:coff + csz],
                            start=(kt == 0), stop=(kt == KT - 1))

                emit_pro(2)

                # ---- weight prep (half-K staging keeps SBUF small),
                #      chunk-ordered: once the dout tiles of a matmul chunk
                #      are done, the first token tiles' matmuls for that
                #      chunk are emitted to keep the PE fed ----
                KH = K // 2
                for dt_i, (doff, p) in enumerate(dout_tiles):
                    for h in range(2):
                        emit_pro(1)
                        ks = slice(h * KH, (h + 1) * KH)
                        wf = wp.tile([128, KH], F32, tag="wf", name="wf")
                        mk = wp.tile([128, KH], U8, tag="mk", name="mk")
                        sgn = wp.tile([128, KH], BF16, tag="sgn", name="sgn")
                        wsb = wp.tile([128, KH], BF16, tag="wsb", name="wsb")
                        nc.scalar.dma_start(wf[:p], w_h[doff:doff + p, ks])
                        nc.scalar.dma_start(mk[:p], mk_h[doff:doff + p, ks])
                        nc.scalar.sign(sgn[:p], wf[:p])
                        # scale multiply on ACT (idle) so DVE only does the
                        # predicated outlier restore + wT copies
                        nc.scalar.mul(wsb[:p], sgn[:p], scale_vec[:p])
                        nc.vector.copy_predicated(wsb[:p], mk[:p], wf[:p])
                        for g4 in range(KH // 512):
                            psw = psum_pool.tile([128, 512], BF16,
                                                 tag="psx", name="psw")
                            for j in range(4):
                                kl = g4 * 4 + j
                                nc.tensor.transpose(
                                    psw[:, j * 128:j * 128 + p],
                                    wsb[:p, kl * 128:(kl + 1) * 128],
                                    identity[:p, :p])
                            kt0 = h * (KH // 128) + g4 * 4
                            nc.vector.tensor_copy(
                                wT_r[:, kt0:kt0 + 4, doff:doff + p],
                                psw.rearrange("a (j c) -> a j c",
                                              j=4)[:, :, :p])
                    # chunk 0 spans dout tiles 0-3, chunk 1 tiles 4-7: stage
                    # the first token tiles' matmuls as chunks become ready
                    # NOTE: emit_mm(t, ci) is only valid once ALL of xT[t]'s
                    # transpose groups have been emitted (pro chunks consumed:
                    # 2 upfront + 2 per dout tile) — a matmul emitted before
                    # its xT writes would read uninitialized SBUF.
                    if tok_tiles >= 2:
                        if dt_i == 3:
                            emit_mm(0, 0)      # xT[0] complete since dt 2
                        elif dt_i == 7:
                            emit_mm(0, 1)
                            emit_mm(1, 0)      # xT[1] complete since dt 6

                emit_pro(len(pro_chunks))

                # bias broadcast deferred to here so it doesn't occupy the
                # gpsimd DMA queue ahead of the first x cast-DMAs
                nc.gpsimd.dma_start(
                    out=bias_rep,
                    in_=b_h[:].rearrange("(a d) -> a d",
                                         a=1).to_broadcast((128, DSH)))

                # ---- steady state ----
                for t in range(tok_tiles):
                    pend = [ci for ci in range(len(CHUNKS))
                            if (t, ci) not in psos_map]
                    psos = {}
                    for ci in pend:
                        psos[ci] = psum_pool.tile([128, 512], F32,
                                                  tag=f"pso{ci}",
                                                  name=f"pso{ci}")
                        psos_map[(t, ci)] = psos[ci]
                    xT_t = xTs[t][0]
                    for kt in range(KT):
                        if t + PRO < tok_tiles:
                            x_transpose(t + PRO, kt, box)
                        for ci in pend:
                            coff, csz = CHUNKS[ci]
                            nc.tensor.matmul(
                                psos[ci][:, :csz],
                                xT_t[:, kt * 128:(kt + 1) * 128],
                                wT_r[:, kt, coff:coff + csz],
                                start=(kt == 0), stop=(kt == KT - 1))
                    xTs.pop(t)
                    osb = xp.tile([128, DSH], F32, tag="osb", name="osb")
                    for ci, (coff, csz) in enumerate(CHUNKS):
                        nc.vector.tensor_add(
                            osb[:, coff:coff + csz],
                            psos_map.pop((t, ci))[:, :csz],
                            bias_rep[:, coff:coff + csz])
                    nc.sync.dma_start(
                        out_h[t * 128:(t + 1) * 128, :], osb)

    _split_excess_waits(nc)
    return nc


_NC_CACHE = {}


def _get_nc(m_tokens: int = M):
    if m_tokens not in _NC_CACHE:
        _NC_CACHE[m_tokens] = build_nc(m_tokens)
    return _NC_CACHE[m_tokens]


def _make_in_maps(x, weight, bias, outlier_mask, binary_scale):
    m_tokens = x.shape[0] * x.shape[1] if x.ndim == 3 else x.shape[0]
    xf = np.ascontiguousarray(x.reshape(m_tokens, K), dtype=np.float32)
    w = np.ascontiguousarray(weight, dtype=np.float32)
    b = np.ascontiguousarray(bias, dtype=np.float32)
    mk = np.ascontiguousarray(outlier_mask).view(np.uint8)
    sc = np.ascontiguousarray(binary_scale, dtype=np.float32).reshape(1, 1)
    ident = np.eye(128, dtype=ml_dtypes.bfloat16)
    in_maps = []
    for i in range(NCORES):
        sl = slice(i * DSH, (i + 1) * DSH)
        in_maps.append({
            "x": xf,
            "weight": np.ascontiguousarray(w[sl]),
            "bias": np.ascontiguousarray(b[sl]),
            "outlier_mask": np.ascontiguousarray(mk[sl]),
            "binary_scale": sc,
            "identity_const": ident,
        })
    return in_maps, m_tokens


def run_sharded(x, weight, bias, outlier_mask, binary_scale, trace=False):
    """Run on 8 cores; returns (full_output [M, DOUT] f32, BassKernelResults)."""
    in_maps, m_tokens = _make_in_maps(x, weight, bias, outlier_mask, binary_scale)
    nc = _get_nc(m_tokens)
    res = run_bass_kernel_spmd(nc, in_maps, core_ids=list(range(NCORES)),
                               trace=trace)
    full = np.concatenate([res.results[i]["out"] for i in range(NCORES)], axis=1)
    return full, res


def kernel(x, weight, bias, outlier_mask, binary_scale):
    full, _ = run_sharded(x, weight, bias, outlier_mask, binary_scale)
    return full.reshape(x.shape[0], x.shape[1], DOUT) if x.ndim == 3 else full

